# revision 4
# baseline (speedup 1.0000x reference)
"""Trainium2 Bass kernel for the Cheirality loss layer (v8, column layout).

Math (per batch b, pixel (y, x); g = grad_dirs, n = normal_flow):
    AV0 = V2*x - V0                    AV1 = V2*y - V1
    BW0 = O0*x*y - O1*(x^2+1) + O2*y   BW1 = O0*(y^2+1) - O1*x*y - O2*x
    dot1 = g0*AV0 + g1*AV1
    gbw  = g0*BW0 + g1*BW1
    rho  = dot1 * (n0 + n1 - gbw)
    out  = mean(gelu(-rho))            (exact erf-based gelu)

Layout (v8): partition p = column-within-group; chunk-pair K covers
columns [128K, 128K+128) of BOTH batches handled by this core.  Tiles are
[128, 2, 480] = (col, batch-half h, row y).  Then x = 128K + p is a
per-partition constant and y varies only along the free axis, shared by
all partitions (ybb tile).  Polynomial regrouping in y:
    gbw  = a0*(g0*y) + b0*g0 + a1*(g1*y) + b1*g1 + c*(g1*ys2)
      a0 = O0*x+O2, b0 = -O1*(x^2+1), a1 = -O1*x, b1 = O0-O2*x,
      ys2 = (y/64)^2, c = 4096*O0      (a*,b*,c per-partition constants)
    dot1 = AV0*g0 + V2*(g1*y) + (-V1)*g1
Device dataflow per chunk-pair:
    DVE: Gy0 = g0*ybb, Gy1 = g1*ybb, Q = g1*ys2bb   (bf16 tensor_tensor)
    PE : NEG  = S*(a0@Gy0 + b0@g0 + a1@Gy1 + b1@g1 + c@Q - nsum)
         D1M  = AV0@g0 + V2@Gy1 + (-V1)@g1
         (diag fp16 weights, per half; NEG coeffs pre-scaled by S=2^-6 on
          host so b0 fits fp16; undone by gelu scale=64)
    DVE: RHO = D1M * NEG  (psum fp32 reads, bf16 out);  -rho*S = RHO
    ACT: gelu(64*RHO) with accum_out -> per-chunk partial sums
    nsum = n0+n1 ships as fp8e4m3 (additive term, |err| ~ 3e-4 relative).
Reduction: accum tile [128, 2*NPAIR] -> host sums in float64.
"""

import numpy as np
import ml_dtypes

import concourse.bacc as bacc
import concourse.bass as bass
import concourse.tile as tile
from concourse import mybir
from concourse.bass_utils import run_bass_kernel_spmd

B, H, W = 16, 480, 640
NCORES = 8
BPC = B // NCORES        # 2 batches per core
NPAIR = W // 128         # 5 column-group chunk-pairs
FH = H                   # 480 free elems per half
FHP = 512                # psum-bank-padded half stride
FF = BPC * FH            # 960 free elems per pair tile
SC = 2.0 ** -6           # NEG pre-scale (host) undone by gelu scale

F32 = mybir.dt.float32
F16 = mybir.dt.float16
BF16 = mybir.dt.bfloat16
FP8 = mybir.dt.float8e4
AF = mybir.ActivationFunctionType
A = mybir.AluOpType

# per-(pair,half) diag slots: a0, b0, a1, b1, av0
PD_A0, PD_B0, PD_A1, PD_B1, PD_AV0 = range(5)
NPD = 5
# per-half diag slots: c, v2, v1n, then shared -S at index 3
HD_C, HD_V2, HD_V1N = range(3)
NHD = 3


def _build_kernel(tc, gd, nsum, ybb, ys2bb, pdiag, hdiag, out):
    nc = tc.nc
    gd_t = gd.ap()
    ns_t = nsum.ap()
    pd_t = pdiag.ap()

    with (
        tc.tile_pool(name="singles", bufs=1) as singles,
        tc.tile_pool(name="ins", bufs=3) as ins,
        tc.tile_pool(name="mids", bufs=2) as mids,
        tc.tile_pool(name="psum", bufs=2, space="PSUM") as psp,
    ):
        yb = singles.tile([128, BPC, FH], F16, name="yb")
        y2b = singles.tile([128, BPC, FH], F16, name="y2b")
        hd = singles.tile([128, BPC, NHD + 1, 128], F16, name="hd")
        acc = singles.tile([128, BPC * NPAIR], F32, name="acc")

        # constant/setup streams: keep the sync queue free for the first
        # gd/nsum chunk loads
        nc.scalar.dma_start(out=yb, in_=ybb.ap())
        nc.scalar.dma_start(out=y2b, in_=ys2bb.ap())
        nc.gpsimd.dma_start(out=hd, in_=hdiag.ap())
        HD = [[hd[:, h, i, :] for i in range(NHD + 1)] for h in range(BPC)]

        for K in range(NPAIR):
            gdt = ins.tile([128, 2, BPC, FH], BF16, tag="gdt", name=f"gd_{K}")
            nst = ins.tile([128, BPC, FH], FP8, tag="nst", name=f"ns_{K}")
            pdt = ins.tile([128, BPC, NPD, 128], F16, tag="pdt", name=f"pd_{K}")
            nc.sync.dma_start(out=gdt, in_=gd_t[K])
            nc.sync.dma_start(out=nst, in_=ns_t[K])
            nc.gpsimd.dma_start(out=pdt, in_=pd_t[K])
            g0 = gdt[:, 0]                       # [128, BPC, FH]
            g1 = gdt[:, 1]

            def mtile(tag, dt=BF16):
                return mids.tile([128, BPC, FH], dt, tag=tag, name=f"{tag}_{K}")

            gy0 = mtile("gy0")
            nc.vector.tensor_tensor(out=gy0, in0=g0, in1=yb, op=A.mult)
            gy1 = mtile("gy1")
            nc.vector.tensor_tensor(out=gy1, in0=g1, in1=yb, op=A.mult)
            q = mtile("q")
            nc.vector.tensor_tensor(out=q, in0=g1, in1=y2b, op=A.mult)

            neg_ps = psp.tile([128, BPC, FHP], F32, tag="neg", name=f"neg_{K}")
            d1_ps = psp.tile([128, BPC, FHP], F32, tag="d1", name=f"d1_{K}")
            for h in range(BPC):
                PD = [pdt[:, h, i, :] for i in range(NPD)]
                neg_terms = [
                    (PD[PD_A0], gy0[:, h]),
                    (PD[PD_B0], g0[:, h]),
                    (PD[PD_A1], gy1[:, h]),
                    (PD[PD_B1], g1[:, h]),
                    (HD[h][HD_C], q[:, h]),
                    (HD[h][NHD], nst[:, h]),
                ]
                for i, (dg, rhs) in enumerate(neg_terms):
                    nc.tensor.matmul(
                        neg_ps[:, h, :FH], dg, rhs,
                        start=(i == 0), stop=(i == len(neg_terms) - 1),
                    )
                d1_terms = [
                    (PD[PD_AV0], g0[:, h]),
                    (HD[h][HD_V2], gy1[:, h]),
                    (HD[h][HD_V1N], g1[:, h]),
                ]
                for i, (dg, rhs) in enumerate(d1_terms):
                    nc.tensor.matmul(
                        d1_ps[:, h, :FH], dg, rhs,
                        start=(i == 0), stop=(i == len(d1_terms) - 1),
                    )

            d1b = mtile("d1b")
            nc.scalar.activation(out=d1b, in_=d1_ps[:, :, :FH], func=AF.Copy)
            rho = mtile("rho")
            nc.vector.tensor_tensor(
                out=rho, in0=d1b, in1=neg_ps[:, :, :FH], op=A.mult
            )
            gl = mtile("gl")
            for h in range(BPC):
                ci = K * BPC + h
                nc.scalar.activation(
                    out=gl[:, h], in_=rho[:, h], func=AF.Gelu,
                    bias=0.0, scale=1.0 / SC,
                    accum_out=acc[:, ci : ci + 1],
                )

        nc.sync.dma_start(out=out.ap(), in_=acc)


def build_bass():
    nc = bacc.Bacc("TRN2", target_bir_lowering=False, debug=False)
    gd = nc.dram_tensor("gd", [NPAIR, 128, 2, BPC, FH], BF16, kind="ExternalInput")
    nsum = nc.dram_tensor("nsum", [NPAIR, 128, BPC, FH], FP8, kind="ExternalInput")
    ybb = nc.dram_tensor("ybb", [128, BPC, FH], F16, kind="ExternalInput")
    ys2bb = nc.dram_tensor("ys2bb", [128, BPC, FH], F16, kind="ExternalInput")
    pdiag = nc.dram_tensor(
        "pdiag", [NPAIR, 128, BPC, NPD, 128], F16, kind="ExternalInput"
    )
    hdiag = nc.dram_tensor(
        "hdiag", [128, BPC, NHD + 1, 128], F16, kind="ExternalInput"
    )
    out = nc.dram_tensor("acc_out", [128, BPC * NPAIR], F32, kind="ExternalOutput")
    with tile.TileContext(nc) as tc:
        _build_kernel(tc, gd, nsum, ybb, ys2bb, pdiag, hdiag, out)
    nc.compile()
    return nc


def make_in_maps(pose, grad_dirs, normal_flow):
    pose = np.asarray(pose, np.float32)
    gd = np.asarray(grad_dirs, np.float32)
    nf = np.asarray(normal_flow, np.float32)

    yv = np.arange(H, dtype=np.float32)
    ybb = np.broadcast_to(yv, (128, BPC, H)).astype(np.float16)
    ys2 = (yv / 64.0) ** 2
    ys2bb = np.broadcast_to(ys2, (128, BPC, H)).astype(np.float16)

    def to_pairs(a):
        # a: [BPC, C, H, W] -> [NPAIR, 128, C, BPC, H]
        Bc, C = a.shape[0], a.shape[1]
        r = a.reshape(Bc, C, H, NPAIR, 128)
        return np.ascontiguousarray(r.transpose(3, 4, 1, 0, 2))

    in_maps = []
    for core in range(NCORES):
        b0 = core * BPC
        gdc = to_pairs(gd[b0 : b0 + BPC]).astype(ml_dtypes.bfloat16)
        nsc = to_pairs(
            (nf[b0 : b0 + BPC, 0] + nf[b0 : b0 + BPC, 1])[:, None]
        )[:, :, 0].astype(ml_dtypes.float8_e4m3fn)

        x = np.arange(W, dtype=np.float32).reshape(NPAIR, 128)
        pdiag = np.zeros((NPAIR, BPC, NPD, 128, 128), np.float16)
        hdiag = np.zeros((BPC, NHD + 1, 128, 128), np.float16)
        for h in range(BPC):
            V = pose[b0 + h, :3]
            O = pose[b0 + h, 3:]
            for K in range(NPAIR):
                xk = x[K]
                coef = np.stack([
                    SC * (O[0] * xk + O[2]),            # a0
                    SC * (-O[1] * (xk * xk + 1.0)),     # b0
                    SC * (-O[1] * xk),                  # a1
                    SC * (O[0] - O[2] * xk),            # b1
                    V[2] * xk - V[0],                   # av0
                ], axis=0).astype(np.float16)
                for i in range(NPD):
                    np.fill_diagonal(pdiag[K, h, i], coef[i])
            hcoef = np.array([
                SC * (4096.0 * O[0]),                   # c
                V[2],                                   # v2
                -V[1],                                  # v1n
                -SC,                                    # nsum coefficient
            ], np.float16)
            for i in range(NHD + 1):
                np.fill_diagonal(hdiag[h, i], hcoef[i])

        # device layout: diag contraction dim on partitions
        pdiag_dev = np.ascontiguousarray(pdiag.transpose(0, 3, 1, 2, 4))
        hdiag_dev = np.ascontiguousarray(hdiag.transpose(2, 0, 1, 3))

        in_maps.append({
            "gd": np.ascontiguousarray(gdc),
            "nsum": np.ascontiguousarray(nsc),
            "ybb": np.ascontiguousarray(ybb),
            "ys2bb": np.ascontiguousarray(ys2bb),
            "pdiag": pdiag_dev,
            "hdiag": hdiag_dev,
        })
    return in_maps


_NC_CACHE = None


def _get_nc():
    global _NC_CACHE
    if _NC_CACHE is None:
        _NC_CACHE = build_bass()
    return _NC_CACHE


def kernel(pose, grad_dirs, normal_flow):
    nc = _get_nc()
    in_maps = make_in_maps(pose, grad_dirs, normal_flow)
    res = run_bass_kernel_spmd(nc, in_maps, core_ids=list(range(NCORES)))
    total = 0.0
    for r in res.results:
        total += r["acc_out"].astype(np.float64).sum()
    return np.float32(total / (B * H * W))
